# revision 1
# baseline (speedup 1.0000x reference)
"""Trainium2 Bass kernel for per-variable gated LoRA mixer (dense_mlp).

Math (reference):
    xr  = x.reshape(b, t, v)                  # b=512, t=512, v=64
    x1  = tanh(gating * xr)
    tmp = einsum('biv,ik->bkv', x1, lora_A)   # r=16
    nx  = einsum('bkv,kov->bov', tmp, lora_B)
    out = xr + nx + bias

Sharding: data-parallel over batch, 64 batch elements per core, 8 cores.
Params (lora_A/lora_B/bias/gating) are replicated, with host-side layout
preparation into matmul-ready tiles.

Per-core device schedule (fp32 everywhere, t on SBUF partitions):
  - xb [128, (b=64, ch=4, v=64)]: x shard, partition p = t within chunk ch
    (t = ch*128 + p). Both the input and output DMAs are 32KB-contiguous
    per (b, ch) block.
  - x1 = tanh(g*x) per batch-octet (8 b's): gpsimd mul + ACT tanh.
  - mm1: lhsT = a_prep [128, (ch, 48)] with cols 0-15 = A, 16-31 = 0,
    32-47 = A -> psum [48, 512] holds tmp twice (rows 0-15 and 32-47).
  - tmp_exp [97, 8192], two 4096-wide regions (vp = 0 / 1):
      rows 0-15 : [tmp | 0]           (k-block of v0 = 2c)
      rows 32-47: [0 | tmp shifted -1] (k-block of v1 = 2c+1)
      row 64    : [1 | 0]   row 96: [0 | 1]   (bias rows)
      all other rows zero.
  - mm2 per v-pair c (v = 2c, 2c+1) and o-chunk q: lhsT = lb_prep slice
    [97, 128] (rows 0-15 = B[:,:,2c], 32-47 = B[:,:,2c+1], 64 = bias[:,2c],
    96 = bias[:,2c+1], rest 0); rhs = tmp_exp AP [97, (vp: stride 4096, 2),
    (b: stride 64, 64)] at offset 2c. The zero/ones regions implement the
    block-diagonal structure and fold the bias in. out psum [128(o),
    (vp, b) = 128].
  - epilogue: DVE tensor_tensor add psum + xb residual -> out tile with
    interleaved (b, v) columns, then DMA out (32KB-contiguous blocks).

Workarounds for this container's walrus build:
  - every instruction may carry at most ONE semaphore wait: TileContext's
    tail drain is patched and a post-pass hoists excess waits onto NoOps.
  - compute-engine APs must start at 32-aligned partitions (hence the
    97-row tmp layout with blocks at 0/32/64/96).
"""

import numpy as np

import concourse.bass as bass
import concourse.mybir as mybir
import concourse.tile as tile
from concourse.bass_utils import run_bass_kernel_spmd

F32 = mybir.dt.float32

N_CORES = 8
B_FULL = 512
T = 512          # window length (= o dim)
V = 64           # n_var
R = 16           # low rank
B = B_FULL // N_CORES   # 64 batch elements per core
NCH = T // 128   # 4 t-chunks
NG = B // 8      # 8 batch-octets
NPAIR = V // 2   # 32 v-pairs
K2 = 97          # contraction rows of mm2 (blocks at 0-15, 32-47, 64, 96)
HB = B * V       # 4096: half-width of tmp_exp


def _patch_tile_tail():
    """Re-emit the kernel-tail Drain's semaphore waits as individual
    wait_ge instructions (walrus here rejects multi-wait instructions)."""
    if getattr(tile.TileContext, "_drain_patched", False):
        return

    def _drain_and_barrier(self, tick_clock, wait_clock):
        nc = self.nc
        from concourse.tile import ScopedClock

        drain_inst = nc.sync.drain()
        wait_clock.add_sem_waits(
            drain_inst.ins, ScopedClock({None: tick_clock.global_clock})
        )
        si = drain_inst.ins.sync_info
        waits = list(si.on_wait) if si is not None else []
        if len(waits) > 1:
            sems_by_name = {s.name: s for s in self.sems.allocated().values()}
            si.on_wait = []
            for w in waits:
                nc.sync.wait_ge(sems_by_name[w.ant_name], w.wait_value)
        nc.all_engine_barrier()
        popped = nc._tile_sem_poison_stack.pop()
        assert popped is self._sem_poison
        nc.clear_and_free_semaphores(list(self.sems.allocated().values()))
        nc.all_engine_barrier()

    tile.TileContext._drain_and_barrier = _drain_and_barrier
    tile.TileContext._drain_patched = True


def _split_multi_waits(nc, limit=1):
    """Hoist excess semaphore waits onto same-engine NoOps inserted just
    before the offending instruction (program order per engine preserves
    the wait-before-execute semantics)."""
    ctr = 0
    for f in nc.m.functions:
        for b in f.blocks:
            insts = list(b.instructions)
            out = []
            changed = False
            for inst in insts:
                si = inst.sync_info
                if si is not None:
                    waits = list(si.on_wait)
                    if len(waits) > limit:
                        for w in waits[:-limit]:
                            nop = mybir.InstNoOp(name=f"zzws_{ctr}")
                            ctr += 1
                            nop.engine = inst.engine
                            nop.sync_info = mybir.SyncInfo(
                                on_wait=[w], on_update=[]
                            )
                            out.append(nop)
                        si.on_wait = waits[-limit:]
                        changed = True
                out.append(inst)
            if changed:
                b.instructions = out
    return ctr


def build_program():
    _patch_tile_tail()
    nc = bass.Bass()

    x_d = nc.dram_tensor("x", [B, T, V], F32, kind="ExternalInput")
    a_d = nc.dram_tensor("a", [128, NCH, 48], F32, kind="ExternalInput")
    lb_d = nc.dram_tensor("lb", [K2, NCH * NPAIR * 128], F32, kind="ExternalInput")
    g_d = nc.dram_tensor("g", [128, V], F32, kind="ExternalInput")
    out_d = nc.dram_tensor("out", [B, T, V], F32, kind="ExternalOutput")

    with tile.TileContext(nc) as tc:
        with (
            tc.tile_pool(name="pers", bufs=1) as pers,
            tc.tile_pool(name="x1p", bufs=2) as x1p,
            tc.tile_pool(name="lbp", bufs=2) as lbp,
            tc.tile_pool(name="outp", bufs=2) as outp,
            tc.tile_pool(name="ps1", bufs=2, space="PSUM") as ps1,
            tc.tile_pool(name="ps2", bufs=4, space="PSUM") as ps2,
        ):
            xb = pers.tile([128, B * NCH * V], F32)        # 64KB/partition
            tmp = pers.tile([K2, 2 * HB], F32)             # [97, 8192]
            a_sb = pers.tile([128, NCH * 48], F32)
            g_sb = pers.tile([128, V], F32)

            # --- input DMAs ---
            nc.sync.dma_start(
                a_sb.rearrange("p (ch k) -> p ch k", ch=NCH), a_d[:, :, :]
            )
            nc.sync.dma_start(g_sb[:, :], g_d[:, :])
            # x: [b, (ch p), v] -> partitions p, cols (b, ch, v).
            # Split per batch-octet so phase 1 starts as soon as the first
            # 1MB lands instead of waiting for the whole 8MB.
            xbv = xb.rearrange("p (b ch v) -> p b ch v", b=B, ch=NCH)
            xsrc = x_d.rearrange("b (ch p) v -> p b ch v", ch=NCH, p=128)
            for g in range(NG):
                nc.sync.dma_start(
                    xbv[:, g * 8 : (g + 1) * 8], xsrc[:, g * 8 : (g + 1) * 8]
                )

            # --- tmp_exp constant regions (32-aligned partition bases) ---
            nc.vector.memset(tmp[:, :], 0.0)
            nc.vector.memset(tmp[64:65, 0:HB], 1.0)
            nc.vector.memset(tmp[96:97, HB:], 1.0)

            # --- phase 1: gate+tanh, mm1, tmp copies (per batch-octet g) ---
            a_sbv = a_sb.rearrange("p (ch k) -> p ch k", ch=NCH)
            for g in range(NG):
                x1_t = x1p.tile([128, 8 * NCH * V], F32)  # [128, 2048]
                x1v = x1_t.rearrange("p (b ch v) -> p b ch v", b=8, ch=NCH)
                xg = xbv[:, g * 8 : (g + 1) * 8]
                gb = g_sb[:, :].unsqueeze(1).unsqueeze(1).broadcast_to(
                    (128, 8, NCH, V)
                )
                nc.gpsimd.tensor_tensor(
                    out=x1v, in0=xg, in1=gb, op=mybir.AluOpType.mult
                )
                nc.scalar.activation(
                    x1_t[:, :], x1_t[:, :], mybir.ActivationFunctionType.Tanh
                )

                p1 = ps1.tile([48, 512], F32)
                x1m = x1_t.rearrange("p (b ch v) -> p ch b v", b=8, ch=NCH)
                for ch in range(NCH):
                    nc.tensor.matmul(
                        p1[:, :],
                        a_sbv[:, ch],
                        x1m[:, ch],
                        start=(ch == 0),
                        stop=(ch == NCH - 1),
                    )
                # block v0 (rows 0-15): tmp columns of this octet
                nc.scalar.copy(tmp[0:R, g * 512 : (g + 1) * 512], p1[0:R, :])
                # block v1 (rows 32-47): tmp shifted one column left, region 1
                nc.vector.tensor_copy(
                    tmp[32:48, HB - 1 + g * 512 : HB - 1 + (g + 1) * 512],
                    p1[32:48, :],
                )

            # --- phase 2: mm2 + epilogue per o-chunk q ---
            tmpv = tmp.rearrange("p (r b v) -> p r b v", r=2, b=B)
            xbe = xb.rearrange(
                "p (b ch o s r) -> p b ch o s r", b=B, ch=NCH, o=8, s=4
            )
            for q in range(NCH):
                lb_t = lbp.tile([K2, NPAIR * 128], F32)
                nc.sync.dma_start(
                    lb_t[:, :], lb_d[:, q * NPAIR * 128 : (q + 1) * NPAIR * 128]
                )
                out_t = outp.tile([128, B * V], F32)
                otv = out_t.rearrange("p (b o s r) -> p b o s r", b=B, o=8, s=4)
                for Q in range(8):  # octet of v (4 pairs)
                    p2 = ps2.tile([128, 512], F32)
                    for j in range(4):
                        c = 4 * Q + j
                        nc.tensor.matmul(
                            p2[:, j * 128 : (j + 1) * 128],
                            lb_t[:, c * 128 : (c + 1) * 128],
                            tmpv[:, :, :, 2 * c],
                            start=True,
                            stop=True,
                        )
                    p2v = p2.rearrange("p (s r b) -> p b s r", s=4, r=2)
                    nc.vector.tensor_tensor(
                        out=otv[:, :, Q],
                        in0=p2v,
                        in1=xbe[:, :, q, Q],
                        op=mybir.AluOpType.add,
                    )
                nc.sync.dma_start(
                    out_d.rearrange("b (q p) v -> p q b v", q=NCH)[:, q],
                    out_t.rearrange("p (b v) -> p b v", b=B),
                )

    n_split = _split_multi_waits(nc)
    print(f"[kernel] wait-split nops inserted: {n_split}")
    return nc


_PROGRAM = None


def _get_program():
    global _PROGRAM
    if _PROGRAM is None:
        _PROGRAM = build_program()
    return _PROGRAM


def _host_prep(gating, bias, lora_A, lora_B):
    # a_prep: [128, ch, 48]; cols 0-15 = A rows of this chunk, 16-31 = 0,
    # 32-47 = A again (tmp lands at psum rows 0-15 and 32-47).
    a_prep = np.zeros((128, NCH, 48), dtype=np.float32)
    a_chunks = lora_A.reshape(NCH, 128, R).transpose(1, 0, 2)  # [128, ch, r]
    a_prep[:, :, 0:R] = a_chunks
    a_prep[:, :, 32:48] = a_chunks

    # lb_prep: [97, q, c, 128]; rows 0-15 = B[:, o, 2c], 32-47 = B[:, o, 2c+1],
    # 64 = bias[o, 2c], 96 = bias[o, 2c+1], rest zero.
    lb_prep = np.zeros((K2, NCH, NPAIR, 128), dtype=np.float32)
    lbr = lora_B.reshape(R, NCH, 128, NPAIR, 2)  # [r, q, o', c, 2]
    lb_prep[0:R] = lbr[:, :, :, :, 0].transpose(0, 1, 3, 2)
    lb_prep[32:48] = lbr[:, :, :, :, 1].transpose(0, 1, 3, 2)
    br = bias.reshape(NCH, 128, NPAIR, 2)
    lb_prep[64] = br[:, :, :, 0].transpose(0, 2, 1)
    lb_prep[96] = br[:, :, :, 1].transpose(0, 2, 1)

    g_prep = np.broadcast_to(gating.astype(np.float32), (128, V)).copy()
    return a_prep, lb_prep.reshape(K2, NCH * NPAIR * 128), g_prep


def kernel(x, gating, bias, lora_A, lora_B):
    x = np.ascontiguousarray(np.asarray(x, dtype=np.float32)).reshape(B_FULL, T, V)
    gating = np.asarray(gating, dtype=np.float32)
    bias = np.ascontiguousarray(np.asarray(bias, dtype=np.float32))
    lora_A = np.ascontiguousarray(np.asarray(lora_A, dtype=np.float32))
    lora_B = np.ascontiguousarray(np.asarray(lora_B, dtype=np.float32))

    a_prep, lb_prep, g_prep = _host_prep(gating, bias, lora_A, lora_B)

    nc = _get_program()
    in_maps = []
    for c in range(N_CORES):
        shard = np.ascontiguousarray(x[c * B : (c + 1) * B])
        in_maps.append({"x": shard, "a": a_prep, "lb": lb_prep, "g": g_prep})
    res = run_bass_kernel_spmd(nc, in_maps, core_ids=list(range(N_CORES)))
    out = np.concatenate([r["out"] for r in res.results], axis=0)
    return out.reshape(B_FULL, T, V, 1)



# revision 5
# speedup vs baseline: 5.4618x; 5.4618x over previous
"""Trainium2 Bass kernel for per-variable gated LoRA mixer (dense_mlp).

Math (reference):
    xr  = x.reshape(b, t, v)                  # b=512, t=512, v=64
    x1  = tanh(gating * xr)
    tmp = einsum('biv,ik->bkv', x1, lora_A)   # r=16
    nx  = einsum('bkv,kov->bov', tmp, lora_B)
    out = xr + nx + bias

Split of work:
  HOST (numpy, fp32): pre-scale xs = gating*x, pre-transpose into the exact
    SBUF images (bf16), and the final out = x + nx + bias (nx from device).
  DEVICE (per core, 64 batch): x1 = tanh(xs) on ACT; mm1 (contract t) and
    mm2 (contract r, 4 vars packed block-diagonally per matmul) on PE in
    bf16; psum evacuation on ACT/DVE; all HBM I/O as large contiguous DMAs.

Device layouts (per core, b = 64 local batch):
  xs   [128, (g8=8, ch=4, j=4, vs=16, bl=8)] bf16: partition p = t within
       chunk ch (t = ch*128+p); g8*8+bl = local batch; v = 4*vs+j.
  a4   [128, (ch=4, 128)] bf16: 8 copies of lora_A chunk cols (16*cblk+k).
  mm1 per octet-pair P (g8 = 2P, 2P+1): psum p1 [128, (gp=2, 512)] fp32;
       rows 32j+k and 32j+16+k both hold tmp[k] (duplicated lhsT blocks)
       so the 32-row aligned copies p1[32j:32j+32] -> tmps[32j:32j+32] are
       legal (walrus requires 32-aligned compute-AP partition starts).
  tmps [128, (j=4, vs=16, g8=8, bl=8)] bf16: rows 32j..32j+15 = tmp k-rows
       of var class j, rows 32j+16..31 = duplicate junk (weights there = 0).
  lbw  [128, (q=4, g=16, o=128)] bf16: rows 32j+k = lora_B[k, q*128+o, 4g+j],
       rows 32j+16..31 = 0.
  mm2 per (q, g): lhsT = lbw slice [128, 128]; rhs = tmps AP [128, (j=4
       stride 1024, b=64)] at base g*64 -> psum [128 (o), (j, b) = 256] bf16,
       4 g's per psum bank; DVE copies bank -> out tile.
  out  [q=4][128, (g=16, j=4, b=64)] bf16 -> 1MB contiguous DMA per q.

Workarounds for this container's walrus build:
  - every instruction may carry at most ONE semaphore wait: TileContext's
    tail drain is patched and a post-pass hoists excess waits onto NoOps.
  - compute-engine APs must start at 32-aligned partitions.
"""

import numpy as np
import ml_dtypes

import concourse.bass as bass
import concourse.mybir as mybir
import concourse.tile as tile
from concourse.bass_utils import run_bass_kernel_spmd

F32 = mybir.dt.float32
BF16 = mybir.dt.bfloat16
BFNP = ml_dtypes.bfloat16

N_CORES = 8
B_FULL = 512
T = 512          # window length (= o dim)
V = 64           # n_var
R = 16           # low rank
B = B_FULL // N_CORES   # 64 batch elements per core
NCH = T // 128   # 4 t-chunks
NG8 = B // 8     # 8 batch-octets
NP = 4           # octet-pairs in phase 1
J = 4            # var classes packed per mm2 matmul
VS = V // J      # 16 var slots per class
XCOLS = B * NCH * V          # 16384
QCOL = XCOLS // 4            # 4096 (one octet-pair of xs columns)


def _patch_tile_tail():
    """Re-emit the kernel-tail Drain's semaphore waits as individual
    wait_ge instructions (walrus here rejects multi-wait instructions)."""
    if getattr(tile.TileContext, "_drain_patched", False):
        return

    def _drain_and_barrier(self, tick_clock, wait_clock):
        nc = self.nc
        from concourse.tile import ScopedClock

        drain_inst = nc.sync.drain()
        wait_clock.add_sem_waits(
            drain_inst.ins, ScopedClock({None: tick_clock.global_clock})
        )
        si = drain_inst.ins.sync_info
        waits = list(si.on_wait) if si is not None else []
        if len(waits) > 1:
            sems_by_name = {s.name: s for s in self.sems.allocated().values()}
            si.on_wait = []
            for w in waits:
                nc.sync.wait_ge(sems_by_name[w.ant_name], w.wait_value)
        nc.all_engine_barrier()
        popped = nc._tile_sem_poison_stack.pop()
        assert popped is self._sem_poison
        nc.clear_and_free_semaphores(list(self.sems.allocated().values()))
        nc.all_engine_barrier()

    tile.TileContext._drain_and_barrier = _drain_and_barrier
    tile.TileContext._drain_patched = True


def _split_multi_waits(nc, limit=1):
    """Hoist excess semaphore waits onto same-engine NoOps inserted just
    before the offending instruction (program order per engine preserves
    the wait-before-execute semantics)."""
    ctr = 0
    for f in nc.m.functions:
        for b in f.blocks:
            insts = list(b.instructions)
            out = []
            changed = False
            for inst in insts:
                si = inst.sync_info
                if si is not None:
                    waits = list(si.on_wait)
                    if len(waits) > limit:
                        for w in waits[:-limit]:
                            nop = mybir.InstNoOp(name=f"zzws_{ctr}")
                            ctr += 1
                            nop.engine = inst.engine
                            nop.sync_info = mybir.SyncInfo(
                                on_wait=[w], on_update=[]
                            )
                            out.append(nop)
                        si.on_wait = waits[-limit:]
                        changed = True
                out.append(inst)
            if changed:
                b.instructions = out
    return ctr


def build_program():
    _patch_tile_tail()
    nc = bass.Bass()

    xs_d = nc.dram_tensor("xs", [128, XCOLS], BF16, kind="ExternalInput")
    a4_d = nc.dram_tensor("a4", [128, NCH * 128], BF16, kind="ExternalInput")
    lbw_d = nc.dram_tensor("lbw", [128, NCH * VS * 128], BF16, kind="ExternalInput")
    out_d = nc.dram_tensor("out", [NCH, 128, VS * J * B], BF16, kind="ExternalOutput")

    with tile.TileContext(nc) as tc:
        with (
            tc.tile_pool(name="pers", bufs=1) as pers,
            tc.tile_pool(name="outp", bufs=2) as outp,
            tc.tile_pool(name="ps1", bufs=2, space="PSUM") as ps1,
            tc.tile_pool(name="ps2", bufs=2, space="PSUM") as ps2,
        ):
            xs = pers.tile([128, XCOLS], BF16)           # 32KB/partition
            a4 = pers.tile([128, NCH * 128], BF16)
            lbw = pers.tile([128, NCH * VS * 128], BF16)
            tmps = pers.tile([128, J * VS * B], BF16)    # [128, 4096]

            # --- input DMAs (xs quartered for overlap; lbw before last) ---
            nc.sync.dma_start(xs[:, 0:QCOL], xs_d[:, 0:QCOL])
            nc.sync.dma_start(a4[:, :], a4_d[:, :])
            nc.sync.dma_start(xs[:, QCOL : 2 * QCOL], xs_d[:, QCOL : 2 * QCOL])
            nc.sync.dma_start(xs[:, 2 * QCOL : 3 * QCOL], xs_d[:, 2 * QCOL : 3 * QCOL])
            nc.sync.dma_start(lbw[:, :], lbw_d[:, :])
            nc.sync.dma_start(xs[:, 3 * QCOL :], xs_d[:, 3 * QCOL :])

            # block-diagonal rhs: off-class regions of tmps must be zero
            nc.vector.memset(tmps[:, :], 0.0)

            # --- phase 1: tanh + mm1 + tmp copies, per octet-pair P ---
            for P in range(NP):
                xq = xs[:, P * QCOL : (P + 1) * QCOL]
                nc.scalar.activation(
                    xq, xq, mybir.ActivationFunctionType.Tanh
                )
                p1 = ps1.tile([128, 2 * 512], F32)       # 2 banks
                for gp in range(2):
                    g8 = 2 * P + gp
                    for ch in range(NCH):
                        nc.tensor.matmul(
                            p1[:, gp * 512 : (gp + 1) * 512],
                            a4[:, ch * 128 : (ch + 1) * 128],
                            xs[:, (g8 * NCH + ch) * 512 : (g8 * NCH + ch + 1) * 512],
                            start=(ch == 0),
                            stop=(ch == NCH - 1),
                        )
                # copies: rows 32j..32j+31 (duplicated tmp), class-j columns
                p1v = p1.rearrange("p (gp j vs b) -> p gp j vs b", gp=2, j=J, vs=VS)
                tv = tmps.rearrange("p (j vs g8 b) -> p g8 j vs b", j=J, vs=VS, g8=NG8)
                for j in range(J):
                    nc.vector.tensor_copy(
                        tv[32 * j : 32 * j + 32, 2 * P : 2 * P + 2, j],
                        p1v[32 * j : 32 * j + 32, :, j],
                    )

            # --- phase 2: mm2 + evac + out DMA, per o-chunk q ---
            tmpv = tmps.rearrange("p (j c) -> p j c", j=J)
            for q in range(NCH):
                out_t = outp.tile([128, VS * J * B], BF16)
                for g4 in range(4):
                    pb = ps2.tile([128, 4 * J * B], F32)    # 2 banks
                    for gg in range(4):
                        g = 4 * g4 + gg
                        nc.tensor.matmul(
                            pb[:, gg * 256 : (gg + 1) * 256],
                            lbw[:, (q * VS + g) * 128 : (q * VS + g + 1) * 128],
                            tmpv[:, :, g * B : (g + 1) * B],
                            start=True,
                            stop=True,
                        )
                    # evac psum -> bf16 out tile, alternating DVE / ACT
                    dst = out_t[:, g4 * 1024 : (g4 + 1) * 1024]
                    if (q * 4 + g4) % 2 == 0:
                        nc.vector.tensor_copy(dst, pb[:, :])
                    else:
                        nc.scalar.copy(dst, pb[:, :])
                nc.sync.dma_start(out_d[q], out_t[:, :])

    n_split = _split_multi_waits(nc)
    print(f"[kernel] wait-split nops inserted: {n_split}")
    return nc


_PROGRAM = None


def _get_program():
    global _PROGRAM
    if _PROGRAM is None:
        _PROGRAM = build_program()
    return _PROGRAM


def _host_prep(x, gating, lora_A, lora_B):
    # xs per core: [128, g8, ch, j, vs, bl] = gating[v] * x[b, t, v]
    # x: [512, 512, 64] -> [c, g8, bl, ch, p, vs, j]
    g2 = gating.reshape(VS, J)  # v = 4*vs + j
    xr = x.reshape(N_CORES, NG8, 8, NCH, 128, VS, J) * g2[None, None, None, None, None]
    xs_all = np.ascontiguousarray(
        xr.transpose(0, 4, 1, 3, 6, 5, 2).astype(BFNP)
    ).reshape(N_CORES, 128, XCOLS)

    # a4: [p, ch, 16*cblk + k] = lora_A[ch*128+p, k], 8 copies over cblk
    ap = lora_A.reshape(NCH, 128, R).transpose(1, 0, 2)  # [p, ch, k]
    a4 = np.ascontiguousarray(
        np.broadcast_to(ap[:, :, None, :], (128, NCH, 8, R)).astype(BFNP)
    ).reshape(128, NCH * 128)

    # lbw: rows 32j+k = B[k, q*128+o, 4g+j]; rows 32j+16.. = 0
    lb = lora_B.reshape(R, NCH, 128, VS, J).transpose(4, 0, 1, 3, 2)  # [j,k,q,g,o]
    lbw = np.zeros((J, 32, NCH, VS, 128), dtype=BFNP)
    lbw[:, :R] = lb.astype(BFNP)
    lbw = lbw.reshape(128, NCH * VS * 128)
    return xs_all, a4, lbw


def kernel(x, gating, bias, lora_A, lora_B):
    x = np.ascontiguousarray(np.asarray(x, dtype=np.float32)).reshape(B_FULL, T, V)
    gating = np.asarray(gating, dtype=np.float32)
    bias = np.asarray(bias, dtype=np.float32)
    lora_A = np.ascontiguousarray(np.asarray(lora_A, dtype=np.float32))
    lora_B = np.ascontiguousarray(np.asarray(lora_B, dtype=np.float32))

    xs_all, a4, lbw = _host_prep(x, gating, lora_A, lora_B)

    nc = _get_program()
    in_maps = []
    for c in range(N_CORES):
        in_maps.append({"xs": xs_all[c], "a4": a4, "lbw": lbw})
    res = run_bass_kernel_spmd(nc, in_maps, core_ids=list(range(N_CORES)))

    out = np.empty((B_FULL, T, V), dtype=np.float32)
    xb = x.reshape(N_CORES, B, T, V)
    for c in range(N_CORES):
        # out_d [q, p, (g, j, b)] -> nx[b, (q, p), (vs=g, j)]
        nx = (
            res.results[c]["out"]
            .reshape(NCH, 128, VS, J, B)
            .transpose(4, 0, 1, 2, 3)
            .astype(np.float32)
            .reshape(B, T, V)
        )
        out[c * B : (c + 1) * B] = xb[c] + nx + bias
    return out.reshape(B_FULL, T, V, 1)


# revision 9
# speedup vs baseline: 6.5076x; 1.1915x over previous
"""Trainium2 Bass kernel for per-variable gated LoRA mixer (dense_mlp).

Math (reference):
    xr  = x.reshape(b, t, v)                  # b=512, t=512, v=64
    x1  = tanh(gating * xr)
    tmp = einsum('biv,ik->bkv', x1, lora_A)   # r=16
    nx  = einsum('bkv,kov->bov', tmp, lora_B)
    out = xr + nx + bias

Split of work:
  HOST (numpy, fp32): pre-scale xs = gating*x, pre-transpose into the exact
    SBUF images (bf16), and the final out = x + nx + bias (nx from device).
  DEVICE (per core, 64 batch): x1 = tanh(xs) on ACT; mm1 (contract t) and
    mm2 (contract r, 4 vars packed block-diagonally per matmul) on PE in
    bf16; psum evacuation on ACT/DVE; all HBM I/O as large contiguous DMAs.

Device layouts (per core, b = 64 local batch):
  xs   [128, (g8=8, ch=4, j=4, vs=16, bl=8)] bf16: partition p = t within
       chunk ch (t = ch*128+p); g8*8+bl = local batch; v = 4*vs+j.
  a4   [128, (ch=4, 128)] bf16: 8 copies of lora_A chunk cols (16*cblk+k).
  mm1 per octet-pair P (g8 = 2P, 2P+1): psum p1 [128, (gp=2, 512)] fp32;
       rows 32j+k and 32j+16+k both hold tmp[k] (duplicated lhsT blocks)
       so the 32-row aligned copies p1[32j:32j+32] -> tmps[32j:32j+32] are
       legal (walrus requires 32-aligned compute-AP partition starts).
  tmps [128, (j=4, vs=16, g8=8, bl=8)] bf16: rows 32j..32j+15 = tmp k-rows
       of var class j, rows 32j+16..31 = duplicate junk (weights there = 0).
  lbw  [128, (q=4, g=16, o=128)] bf16: rows 32j+k = lora_B[k, q*128+o, 4g+j],
       rows 32j+16..31 = 0.
  mm2 per (q, g): lhsT = lbw slice [128, 128]; rhs = tmps AP [128, (j=4
       stride 1024, b=64)] at base g*64 -> psum [128 (o), (j, b) = 256] bf16,
       4 g's per psum bank; DVE copies bank -> out tile.
  out  [q=4][128, (g=16, j=4, b=64)] bf16 -> 1MB contiguous DMA per q.

Workarounds for this container's walrus build:
  - every instruction may carry at most ONE semaphore wait: TileContext's
    tail drain is patched and a post-pass hoists excess waits onto NoOps.
  - compute-engine APs must start at 32-aligned partitions.
"""

import numpy as np
import ml_dtypes

import concourse.bass as bass
import concourse.mybir as mybir
import concourse.tile as tile
from concourse.bass_utils import run_bass_kernel_spmd

F32 = mybir.dt.float32
BF16 = mybir.dt.bfloat16
BFNP = ml_dtypes.bfloat16

N_CORES = 8
B_FULL = 512
T = 512          # window length (= o dim)
V = 64           # n_var
R = 16           # low rank
B = B_FULL // N_CORES   # 64 batch elements per core
NCH = T // 128   # 4 t-chunks
NG8 = B // 8     # 8 batch-octets
NP = 4           # octet-pairs in phase 1
J = 4            # var classes packed per mm2 matmul
VS = V // J      # 16 var slots per class
XCOLS = B * NCH * V          # 16384
QCOL = XCOLS // 4            # 4096 (one octet-pair of xs columns)


def _patch_tile_tail():
    """Re-emit the kernel-tail Drain's semaphore waits as individual
    wait_ge instructions (walrus here rejects multi-wait instructions)."""
    if getattr(tile.TileContext, "_drain_patched", False):
        return

    def _drain_and_barrier(self, tick_clock, wait_clock):
        nc = self.nc
        from concourse.tile import ScopedClock

        drain_inst = nc.sync.drain()
        wait_clock.add_sem_waits(
            drain_inst.ins, ScopedClock({None: tick_clock.global_clock})
        )
        si = drain_inst.ins.sync_info
        waits = list(si.on_wait) if si is not None else []
        if len(waits) > 1:
            sems_by_name = {s.name: s for s in self.sems.allocated().values()}
            si.on_wait = []
            for w in waits:
                nc.sync.wait_ge(sems_by_name[w.ant_name], w.wait_value)
        nc.all_engine_barrier()
        popped = nc._tile_sem_poison_stack.pop()
        assert popped is self._sem_poison
        nc.clear_and_free_semaphores(list(self.sems.allocated().values()))
        nc.all_engine_barrier()

    tile.TileContext._drain_and_barrier = _drain_and_barrier
    tile.TileContext._drain_patched = True


def _split_multi_waits(nc, limit=1):
    """Hoist excess semaphore waits onto same-engine NoOps inserted just
    before the offending instruction (program order per engine preserves
    the wait-before-execute semantics)."""
    ctr = 0
    for f in nc.m.functions:
        for b in f.blocks:
            insts = list(b.instructions)
            out = []
            changed = False
            for inst in insts:
                si = inst.sync_info
                if si is not None:
                    waits = list(si.on_wait)
                    if len(waits) > limit:
                        for w in waits[:-limit]:
                            nop = mybir.InstNoOp(name=f"zzws_{ctr}")
                            ctr += 1
                            nop.engine = inst.engine
                            nop.sync_info = mybir.SyncInfo(
                                on_wait=[w], on_update=[]
                            )
                            out.append(nop)
                        si.on_wait = waits[-limit:]
                        changed = True
                out.append(inst)
            if changed:
                b.instructions = out
    return ctr


def build_program():
    _patch_tile_tail()
    nc = bass.Bass()

    xs_d = nc.dram_tensor("xs", [128, XCOLS], BF16, kind="ExternalInput")
    a4_d = nc.dram_tensor("a4", [128, NCH * 128], BF16, kind="ExternalInput")
    lbw_d = nc.dram_tensor("lbw", [128, NCH * VS * 128], BF16, kind="ExternalInput")
    out_d = nc.dram_tensor("out", [NCH, 128, VS * J * B], BF16, kind="ExternalOutput")

    with tile.TileContext(nc) as tc:
        with (
            tc.tile_pool(name="pers", bufs=1) as pers,
            tc.tile_pool(name="outp", bufs=2) as outp,
            tc.tile_pool(name="ps1", bufs=2, space="PSUM") as ps1,
            tc.tile_pool(name="ps2", bufs=2, space="PSUM") as ps2,
        ):
            xs = pers.tile([128, XCOLS], BF16)           # 32KB/partition
            a4 = pers.tile([128, NCH * 128], BF16)
            lbw = pers.tile([128, NCH * VS * 128], BF16)
            tmps = pers.tile([128, J * VS * B], BF16)    # [128, 4096]

            # --- input DMAs (xs quartered for overlap; lbw before last) ---
            nc.sync.dma_start(xs[:, 0:QCOL], xs_d[:, 0:QCOL])
            nc.sync.dma_start(a4[:, :], a4_d[:, :])
            nc.sync.dma_start(xs[:, QCOL : 2 * QCOL], xs_d[:, QCOL : 2 * QCOL])
            nc.sync.dma_start(xs[:, 2 * QCOL : 3 * QCOL], xs_d[:, 2 * QCOL : 3 * QCOL])
            nc.sync.dma_start(xs[:, 3 * QCOL :], xs_d[:, 3 * QCOL :])
            nc.sync.dma_start(lbw[:, :], lbw_d[:, :])

            # block-diagonal rhs: off-class regions of tmps must be zero
            nc.gpsimd.memset(tmps[:, :], 0.0)

            # --- phase 1: tanh + mm1 + tmp copies, per octet-pair P ---
            for P in range(NP):
                xq = xs[:, P * QCOL : (P + 1) * QCOL]
                nc.scalar.activation(
                    xq, xq, mybir.ActivationFunctionType.Tanh
                )
                p1 = ps1.tile([128, 2 * 512], F32)       # 2 banks
                for gp in range(2):
                    g8 = 2 * P + gp
                    for ch in range(NCH):
                        nc.tensor.matmul(
                            p1[:, gp * 512 : (gp + 1) * 512],
                            a4[:, ch * 128 : (ch + 1) * 128],
                            xs[:, (g8 * NCH + ch) * 512 : (g8 * NCH + ch + 1) * 512],
                            start=(ch == 0),
                            stop=(ch == NCH - 1),
                        )
                # copies: rows 32j..32j+31 (duplicated tmp), class-j columns
                p1v = p1.rearrange("p (gp j vs b) -> p gp j vs b", gp=2, j=J, vs=VS)
                tv = tmps.rearrange("p (vs j g8 b) -> p g8 j vs b", vs=VS, j=J, g8=NG8)
                for j in range(J):
                    nc.vector.tensor_copy(
                        tv[32 * j : 32 * j + 32, 2 * P : 2 * P + 2, j],
                        p1v[32 * j : 32 * j + 32, :, j],
                    )

            # --- phase 2: mm2 + evac + out DMA, per o-chunk q ---
            tmpv = tmps.rearrange("p (g c) -> p g c", g=VS)
            for q in range(NCH):
                out_t = outp.tile([128, VS * J * B], BF16)
                for g4 in range(4):
                    pb = ps2.tile([128, 4 * J * B], F32)    # 2 banks
                    for gg in range(4):
                        g = 4 * g4 + gg
                        nc.tensor.matmul(
                            pb[:, gg * 256 : (gg + 1) * 256],
                            lbw[:, (q * VS + g) * 128 : (q * VS + g + 1) * 128],
                            tmpv[:, g],
                            start=True,
                            stop=True,
                        )
                    # evac psum -> bf16 out tile, alternating DVE / ACT
                    dst = out_t[:, g4 * 1024 : (g4 + 1) * 1024]
                    if (q * 4 + g4) % 2 == 0:
                        nc.vector.tensor_copy(dst, pb[:, :])
                    else:
                        nc.scalar.copy(dst, pb[:, :])
                nc.sync.dma_start(out_d[q], out_t[:, :])

    n_split = _split_multi_waits(nc)
    print(f"[kernel] wait-split nops inserted: {n_split}")
    return nc


_PROGRAM = None


def _get_program():
    global _PROGRAM
    if _PROGRAM is None:
        _PROGRAM = build_program()
    return _PROGRAM


def _host_prep(x, gating, lora_A, lora_B):
    # xs per core: [128, g8, ch, j, vs, bl] = gating[v] * x[b, t, v]
    # x: [512, 512, 64] -> [c, g8, bl, ch, p, vs, j]
    g2 = gating.reshape(VS, J)  # v = 4*vs + j
    xr = x.reshape(N_CORES, NG8, 8, NCH, 128, VS, J) * g2[None, None, None, None, None]
    xs_all = np.ascontiguousarray(
        xr.transpose(0, 4, 1, 3, 6, 5, 2).astype(BFNP)
    ).reshape(N_CORES, 128, XCOLS)

    # a4: [p, ch, 16*cblk + k] = lora_A[ch*128+p, k], 8 copies over cblk
    ap = lora_A.reshape(NCH, 128, R).transpose(1, 0, 2)  # [p, ch, k]
    a4 = np.ascontiguousarray(
        np.broadcast_to(ap[:, :, None, :], (128, NCH, 8, R)).astype(BFNP)
    ).reshape(128, NCH * 128)

    # lbw: rows 32j+k = B[k, q*128+o, 4g+j]; rows 32j+16.. = 0
    lb = lora_B.reshape(R, NCH, 128, VS, J).transpose(4, 0, 1, 3, 2)  # [j,k,q,g,o]
    lbw = np.zeros((J, 32, NCH, VS, 128), dtype=BFNP)
    lbw[:, :R] = lb.astype(BFNP)
    lbw = lbw.reshape(128, NCH * VS * 128)
    return xs_all, a4, lbw


def kernel(x, gating, bias, lora_A, lora_B):
    x = np.ascontiguousarray(np.asarray(x, dtype=np.float32)).reshape(B_FULL, T, V)
    gating = np.asarray(gating, dtype=np.float32)
    bias = np.asarray(bias, dtype=np.float32)
    lora_A = np.ascontiguousarray(np.asarray(lora_A, dtype=np.float32))
    lora_B = np.ascontiguousarray(np.asarray(lora_B, dtype=np.float32))

    xs_all, a4, lbw = _host_prep(x, gating, lora_A, lora_B)

    nc = _get_program()
    in_maps = []
    for c in range(N_CORES):
        in_maps.append({"xs": xs_all[c], "a4": a4, "lbw": lbw})
    res = run_bass_kernel_spmd(nc, in_maps, core_ids=list(range(N_CORES)))

    out = np.empty((B_FULL, T, V), dtype=np.float32)
    xb = x.reshape(N_CORES, B, T, V)
    for c in range(N_CORES):
        # out_d [q, p, (g, j, b)] -> nx[b, (q, p), (vs=g, j)]
        nx = (
            res.results[c]["out"]
            .reshape(NCH, 128, VS, J, B)
            .transpose(4, 0, 1, 2, 3)
            .astype(np.float32)
            .reshape(B, T, V)
        )
        out[c * B : (c + 1) * B] = xb[c] + nx + bias
    return out.reshape(B_FULL, T, V, 1)
